# revision 4
# baseline (speedup 1.0000x reference)
"""Guided filter (r=40, eps=1e-3) on 8 Trainium2 NeuronCores.

Sharding: pure data-parallel over the batch dim (8 batches -> 8 cores).
Each core processes 3 channel-images of 512x512.

v2 design:
  box2d(x) = box_w via a single DVE tensor_tensor_scan (telescoped sliding
  window: S[i] = S[i-1] + xp[i+81] - xp[i] over a zero-padded row, which
  telescopes to the exact 81-wide window sum; fp32 carry, bf16 out), then
  box_h via a banded bf16 matmul on the TensorEngine that also transposes
  (contraction over the partition dim). Normalization 1/n_h is folded into
  the band (bf16, consistent across tensors); 1/n_w is applied per-partition
  in the transposed space (exact f32) via ACT scale / STT scalar. eps is
  baked into the II box output via a rank-1 matmul of eps*n_w.

  Stage-2 elementwise runs mostly in bf16 (pointwise rounding noise is
  annihilated by the second box filter), spread across ACT (PSUM drains
  with scale), DVE (scans, STTs from PSUM, bf16 4x ops).
"""

import sys
import numpy as np
import ml_dtypes
from contextlib import ExitStack

sys.path.insert(0, "/opt/trn_rl_repo")

import concourse.bass as bass
import concourse.tile as tile
from concourse import bacc, mybir
from concourse.bass_utils import run_bass_kernel_spmd

F32 = mybir.dt.float32
BF16 = mybir.dt.bfloat16
ALU = mybir.AluOpType

R = 40
K = 2 * R + 1          # 81
EPS = 1e-3
HW_ = 512
NB = 4                 # 128-row blocks per axis
CH = 3
P = 128
NCORES = 8
PW = 600               # padded row: 41 zeros | 512 data | 47 zeros
LPAD = R + 1           # 41
SCW = NB * PW - K      # flat scan width (covers all 4 blocks)


def _band_range(c):
    n0 = max(0, P * c - R)
    n1 = min(HW_, P * c + P + R)
    return n0, n1


def make_consts():
    idx = np.arange(HW_)
    n1d = (np.minimum(idx + R, HW_ - 1) - np.maximum(idx - R, 0) + 1).astype(
        np.float64)
    inv_n = (1.0 / n1d)

    mask = (np.abs(idx[:, None] - idx[None, :]) <= R)
    band = (mask * inv_n[None, :]).astype(ml_dtypes.bfloat16)
    # [512 k, 512 o] -> [128 kp, 4 kb * 512 o]
    band = np.ascontiguousarray(
        band.reshape(NB, P, HW_).transpose(1, 0, 2).reshape(P, NB * HW_))

    inv_t = np.ascontiguousarray(
        inv_n.reshape(NB, P).T.astype(np.float32))          # [128, 4]
    epsn = (EPS * n1d).astype(ml_dtypes.bfloat16).reshape(1, HW_)
    ones = np.ones((1, HW_), dtype=ml_dtypes.bfloat16)
    return {"band": band, "inv_n": inv_t, "epsn": epsn, "ones": ones}


def _img_view(dram_ap, c):
    # [3, 512, 512] DRAM tensor -> channel c as [128 hp, 4 hb, 512 w]
    return dram_ap[c].rearrange("(hb hp) w -> hp hb w", hp=P)


def build_model():
    nc = bacc.Bacc("TRN2", target_bir_lowering=False, debug=False,
                   num_devices=NCORES)
    I_d = nc.dram_tensor("I", [CH, HW_, HW_], F32, kind="ExternalInput").ap()
    p_d = nc.dram_tensor("p", [CH, HW_, HW_], F32, kind="ExternalInput").ap()
    band_d = nc.dram_tensor("band", [P, NB * HW_], BF16,
                            kind="ExternalInput").ap()
    invn_d = nc.dram_tensor("inv_n", [P, NB], F32, kind="ExternalInput").ap()
    epsn_d = nc.dram_tensor("epsn", [1, HW_], BF16, kind="ExternalInput").ap()
    ones_d = nc.dram_tensor("ones", [1, HW_], BF16, kind="ExternalInput").ap()
    out_d = nc.dram_tensor("out", [CH, HW_, HW_], F32, kind="ExternalOutput").ap()

    with tile.TileContext(nc) as tc:
        with ExitStack() as ctx:
            build_kernel(ctx, tc, I_d, p_d, out_d, band_d, invn_d, epsn_d,
                         ones_d)
    nc.compile()
    return nc


def build_kernel(ctx, tc, I_d, p_d, out_d, band_d, invn_d, epsn_d, ones_d):
    nc = tc.nc

    consts = ctx.enter_context(tc.tile_pool(name="consts", bufs=1))
    band = consts.tile_from(band_d)
    inv_n = consts.tile_from(invn_d)
    epsn = consts.tile_from(epsn_d)
    ones = consts.tile_from(ones_d)

    pIf = ctx.enter_context(tc.tile_pool(name="If", bufs=2))
    pPf = ctx.enter_context(tc.tile_pool(name="Pf", bufs=2))
    pBf = ctx.enter_context(tc.tile_pool(name="bf", bufs=1))
    pS = ctx.enter_context(tc.tile_pool(name="scn", bufs=1))
    pM = ctx.enter_context(tc.tile_pool(name="mean", bufs=1))
    pT = ctx.enter_context(tc.tile_pool(name="tmp", bufs=1))
    pOut = ctx.enter_context(tc.tile_pool(name="outp", bufs=2))
    pQ2 = ctx.enter_context(tc.tile_pool(name="psq2", bufs=2, space="PSUM"))
    pQ1 = ctx.enter_context(tc.tile_pool(name="psq1", bufs=1, space="PSUM"))
    pR = ctx.enter_context(tc.tile_pool(name="psr", bufs=1, space="PSUM"))

    # persistent padded bf16 tiles; pads zeroed once, interiors rewritten
    Ib = pBf.tile([P, NB, PW], BF16, tag="Ib")
    pb = pBf.tile([P, NB, PW], BF16, tag="pb")
    Ipb = pBf.tile([P, NB, PW], BF16, tag="Ipb")
    IIb = pBf.tile([P, NB, PW], BF16, tag="IIb")
    apad = pBf.tile([P, NB, PW], BF16, tag="apad")
    bpad = pBf.tile([P, NB, PW], BF16, tag="bpad")
    for t in (Ib, pb, Ipb, IIb, apad, bpad):
        nc.gpsimd.memset(t[:, :, 0:LPAD], 0.0)
        nc.gpsimd.memset(t[:, :, LPAD + HW_:PW], 0.0)

    def interior(t):
        return t[:, :, LPAD:LPAD + HW_]

    def flat(t):
        return t[:].rearrange("p hb w -> p (hb w)")

    def scan(dst, src, eng):
        f = flat(src)
        eng.tensor_tensor_scan(dst[:], f[:, K:K + SCW], f[:, 0:SCW], 0.0,
                               op0=ALU.add, op1=ALU.subtract)

    def vpass(S, psum_tile, i, add_eps=False):
        """Banded V-pass for output chunk i: box over partition dim +
        transpose. S is a flat [128, 2400-ish] scan tile."""
        for j in range(NB):
            n0, n1 = _band_range(j)
            nc.tensor.matmul(
                psum_tile[:, n0:n1],
                lhsT=S[:, j * PW + i * P: j * PW + i * P + P],
                rhs=band[:, j * HW_ + n0: j * HW_ + n1],
                start=(j == 0), stop=(j == NB - 1) and not add_eps)
        if add_eps:
            nc.tensor.matmul(
                psum_tile[:, :],
                lhsT=epsn[0:1, i * P:(i + 1) * P],
                rhs=ones[0:1, :],
                start=False, stop=True)

    for c in range(CH):
        I_f = pIf.tile([P, NB, PW], F32, tag="If")
        p_f = pPf.tile([P, NB, PW], F32, tag="pf")
        nc.sync.dma_start(interior(I_f), _img_view(I_d, c))
        nc.sync.dma_start(interior(p_f), _img_view(p_d, c))

        # bf16 conversions (ACT) + products (DVE 4x)
        nc.scalar.copy(interior(Ib), interior(I_f))
        nc.scalar.copy(interior(pb), interior(p_f))
        nc.vector.tensor_mul(interior(Ipb), interior(Ib), interior(pb))
        nc.vector.tensor_mul(interior(IIb), interior(Ib), interior(Ib))

        # box_w scans (fp32 carry, bf16 out)
        SI = pS.tile([P, SCW], BF16, tag="SI")
        Sp = pS.tile([P, SCW], BF16, tag="Sp")
        SIp = pS.tile([P, SCW], BF16, tag="SIp")
        SII = pS.tile([P, SCW], BF16, tag="SII")
        scan(SI, Ib, nc.vector)
        scan(Sp, pb, nc.vector)
        scan(SIp, Ipb, nc.vector)
        scan(SII, IIb, nc.vector)

        # stage 2: V-pass per w-chunk + elementwise
        mIb = pM.tile([P, NB * HW_], BF16, tag="mIb")
        mpb = pM.tile([P, NB * HW_], BF16, tag="mpb")
        cov = pM.tile([P, NB * HW_], F32, tag="cov")
        den = pM.tile([P, NB * HW_], F32, tag="den")
        for i in range(NB):
            qI = pQ2.tile([P, HW_], F32, tag="qI")
            qp = pQ2.tile([P, HW_], F32, tag="qp")
            qIp = pQ1.tile([P, HW_], F32, tag="qIp")
            qII = pQ1.tile([P, HW_], F32, tag="qII")
            vpass(SI[:], qI, i)
            vpass(Sp[:], qp, i)
            vpass(SIp[:], qIp, i)
            vpass(SII[:], qII, i, add_eps=True)

            s = inv_n[:, i:i + 1]
            sl = slice(i * HW_, (i + 1) * HW_)
            nc.scalar.mul(mIb[:, sl], qI[:], s)
            nc.scalar.mul(mpb[:, sl], qp[:], s)
            u = pT.tile([P, HW_], BF16, tag="u")
            v = pT.tile([P, HW_], BF16, tag="v")
            nc.vector.tensor_mul(u[:], mIb[:, sl], mpb[:, sl])
            nc.vector.scalar_tensor_tensor(
                cov[:, sl], qIp[:], s, u[:], op0=ALU.mult, op1=ALU.subtract)
            nc.vector.tensor_mul(v[:], mIb[:, sl], mIb[:, sl])
            nc.vector.scalar_tensor_tensor(
                den[:, sl], qII[:], s, v[:], op0=ALU.mult, op1=ALU.subtract)

        rcp = pM.tile([P, NB * HW_], F32, tag="rcp")
        nc.vector.reciprocal_approx_fast(rcp[:], den[:])
        nc.vector.tensor_mul(
            interior(apad).rearrange("p hb w -> p (hb w)"), cov[:], rcp[:])
        t2 = pT.tile([P, NB * HW_], BF16, tag="t2")
        nc.vector.tensor_mul(
            t2[:], interior(apad).rearrange("p hb w -> p (hb w)"), mIb[:])
        nc.vector.tensor_sub(
            interior(bpad).rearrange("p hb w -> p (hb w)"), mpb[:], t2[:])

        # stage 3: box2d of a, b + final combine
        Sa = pS.tile([P, SCW], BF16, tag="Sa")
        Sb = pS.tile([P, SCW], BF16, tag="Sb")
        scan(Sa, apad, nc.vector)
        scan(Sb, bpad, nc.vector)

        out_t = pOut.tile([P, NB, HW_], F32, tag="out")
        for j in range(NB):
            ra = pR.tile([P, HW_], F32, tag="ra")
            rb = pR.tile([P, HW_], F32, tag="rb")
            vpass(Sa[:], ra, j)
            vpass(Sb[:], rb, j)
            sh = inv_n[:, j:j + 1]
            f1 = pT.tile([P, HW_], F32, tag="f1")
            nc.vector.scalar_tensor_tensor(
                f1[:], ra[:], sh, I_f[:, j, LPAD:LPAD + HW_],
                op0=ALU.mult, op1=ALU.mult)
            nc.vector.scalar_tensor_tensor(
                out_t[:, j, :], rb[:], sh, f1[:],
                op0=ALU.mult, op1=ALU.add)

        nc.sync.dma_start(_img_view(out_d, c), out_t[:])


_NC_CACHE = None
LAST_RESULT = None


def _get_model():
    global _NC_CACHE
    if _NC_CACHE is None:
        _NC_CACHE = build_model()
    return _NC_CACHE


def kernel(I, p, _trace=False):
    global LAST_RESULT
    I = np.asarray(I, dtype=np.float32)
    p = np.asarray(p, dtype=np.float32)
    B = I.shape[0]
    assert I.shape == (B, CH, HW_, HW_), I.shape
    nc = _get_model()
    consts = make_consts()
    in_maps = []
    for k in range(NCORES):
        m = {"I": np.ascontiguousarray(I[k]), "p": np.ascontiguousarray(p[k])}
        m.update(consts)
        in_maps.append(m)
    res = run_bass_kernel_spmd(nc, in_maps, core_ids=list(range(NCORES)),
                               trace=_trace)
    LAST_RESULT = res
    out = np.stack([res.results[k]["out"] for k in range(NCORES)], axis=0)
    return out.astype(np.float32)


if __name__ == "__main__":
    rng = np.random.default_rng(0)
    I = rng.random((8, CH, HW_, HW_), dtype=np.float32)
    p = rng.random((8, CH, HW_, HW_), dtype=np.float32)
    out = kernel(I, p)
    print("out", out.shape, out.dtype, float(out.mean()))


# revision 6
# speedup vs baseline: 1.7422x; 1.7422x over previous
"""Guided filter (r=40, eps=1e-3) on 8 Trainium2 NeuronCores.

Sharding: pure data-parallel over the batch dim (8 batches -> 8 cores).
Each core processes 3 channel-images of 512x512.

v4 design (strided stage-2):
  a = cov/(var+eps) and b are pointwise functions of 81x81-box-filtered
  fields, so they are smooth on the r=40 scale. Stage 1 computes the four
  box sums (I, p, Ip, II) over EXACT full-res windows but only at a
  stride-4 sample grid (h,w in {4k+2}), via two banded bf16 indicator
  matmuls (box over the partition dim + transpose each time; 0/1 band is
  exact in bf16). Stage 2 normalizes with an exact f32 normC = 1/(nh*nw)
  and runs the elementwise math in f32 on tiny [128,128] tiles.
  Stage 3 evaluates mean_a/mean_b DENSELY from the strided a,b samples:
  one banded matmul per direction whose 0/1 matrix is the 81-window
  indicator on the sample grid (W1), then one with 1/n4(w) folded (W2,
  bf16); the h-direction sample count 1/n4(h) is applied exactly in f32
  as the ACT drain scale. Final combine: out = mean_a*I + mean_b on DVE.

  PSUM->SBUF traffic is ~15 chunk-drains/image (vs 48 for the full-res
  all-matmul design); TensorE work is ~9k cols/image. The kernel is
  memory-bound (~10 MB/core HBM traffic).
"""

import sys
import numpy as np
import ml_dtypes
from contextlib import ExitStack

sys.path.insert(0, "/opt/trn_rl_repo")

import concourse.bass as bass
import concourse.tile as tile
from concourse import bacc, mybir
from concourse.bass_utils import run_bass_kernel_spmd

F32 = mybir.dt.float32
BF16 = mybir.dt.bfloat16
ALU = mybir.AluOpType

R = 40
EPS = 1e-3
HW_ = 512
NB = 4
CH = 3
P = 128
NCORES = 8
S4 = 4
OFF = 2
NS = HW_ // S4          # 128 samples per axis


def _samp_range(j):
    # samples s with grid(s)=4s+OFF within [128j-40, 128j+127+40]
    s0 = max(0, -(-(P * j - R - OFF) // S4))
    s1 = min(NS, (P * j + P - 1 + R - OFF) // S4 + 1)
    return s0, s1


def make_consts():
    idx = np.arange(HW_)
    n1d = (np.minimum(idx + R, HW_ - 1) - np.maximum(idx - R, 0) + 1).astype(
        np.float64)
    grid = np.arange(NS) * S4 + OFF

    # 0/1 band: bandS[k, j*NS+s] = 1 if |(j*128+k) - grid(s)| <= R
    kk = np.arange(P)
    bandS = np.zeros((P, NB * NS), dtype=ml_dtypes.bfloat16)
    for j in range(NB):
        m = (np.abs((j * P + kk)[:, None] - grid[None, :]) <= R)
        bandS[:, j * NS:(j + 1) * NS] = m.astype(ml_dtypes.bfloat16)

    normC = (1.0 / (n1d[grid][:, None] * n1d[grid][None, :])).astype(np.float32)

    W_ind = (np.abs(grid[:, None] - idx[None, :]) <= R)
    n4 = W_ind.sum(axis=0).astype(np.float64)
    W1 = W_ind.astype(ml_dtypes.bfloat16)
    W2 = (W_ind * (1.0 / n4)[None, :]).astype(ml_dtypes.bfloat16)
    invn4 = np.ascontiguousarray((1.0 / n4).reshape(NB, P).T.astype(np.float32))
    return {"bandS": np.ascontiguousarray(bandS), "normC": normC,
            "W1": np.ascontiguousarray(W1), "W2": np.ascontiguousarray(W2),
            "invn4": invn4}


def _img_view(dram_ap, c):
    return dram_ap[c].rearrange("(hb hp) w -> hp hb w", hp=P)


def build_model():
    nc = bacc.Bacc("TRN2", target_bir_lowering=False, debug=False,
                   num_devices=NCORES)
    I_d = nc.dram_tensor("I", [CH, HW_, HW_], F32, kind="ExternalInput").ap()
    p_d = nc.dram_tensor("p", [CH, HW_, HW_], F32, kind="ExternalInput").ap()
    bandS_d = nc.dram_tensor("bandS", [P, NB * NS], BF16,
                             kind="ExternalInput").ap()
    normC_d = nc.dram_tensor("normC", [NS, NS], F32, kind="ExternalInput").ap()
    W1_d = nc.dram_tensor("W1", [NS, HW_], BF16, kind="ExternalInput").ap()
    W2_d = nc.dram_tensor("W2", [NS, HW_], BF16, kind="ExternalInput").ap()
    invn4_d = nc.dram_tensor("invn4", [P, NB], F32, kind="ExternalInput").ap()
    out_d = nc.dram_tensor("out", [CH, HW_, HW_], F32,
                           kind="ExternalOutput").ap()

    with tile.TileContext(nc) as tc:
        with ExitStack() as ctx:
            build_kernel(ctx, tc, I_d, p_d, out_d, bandS_d, normC_d, W1_d,
                         W2_d, invn4_d)
    nc.compile()
    return nc


def build_kernel(ctx, tc, I_d, p_d, out_d, bandS_d, normC_d, W1_d, W2_d,
                 invn4_d):
    nc = tc.nc

    consts = ctx.enter_context(tc.tile_pool(name="consts", bufs=1))
    bandS = consts.tile_from(bandS_d)
    normC = consts.tile_from(normC_d)
    W1 = consts.tile_from(W1_d)
    W2 = consts.tile_from(W2_d)
    invn4 = consts.tile_from(invn4_d)

    pIf = ctx.enter_context(tc.tile_pool(name="If", bufs=2))
    pPf = ctx.enter_context(tc.tile_pool(name="Pf", bufs=2))
    pBf = ctx.enter_context(tc.tile_pool(name="bfp", bufs=2))
    pYb = ctx.enter_context(tc.tile_pool(name="ybp", bufs=2))
    pS2 = ctx.enter_context(tc.tile_pool(name="s2p", bufs=2))
    pT1 = ctx.enter_context(tc.tile_pool(name="t1p", bufs=2))
    pMa = ctx.enter_context(tc.tile_pool(name="map", bufs=2))
    pOut = ctx.enter_context(tc.tile_pool(name="outp", bufs=2))
    pY = ctx.enter_context(tc.tile_pool(name="psy", bufs=2, space="PSUM"))
    pQ = ctx.enter_context(tc.tile_pool(name="psq", bufs=1, space="PSUM"))
    pO1 = ctx.enter_context(tc.tile_pool(name="pso1", bufs=1, space="PSUM"))
    pRR = ctx.enter_context(tc.tile_pool(name="psrr", bufs=1, space="PSUM"))

    for c in range(CH):
        I_f = pIf.tile([P, NB, HW_], F32, tag="If")
        p_f = pPf.tile([P, NB, HW_], F32, tag="pf")
        nc.sync.dma_start(I_f[:], _img_view(I_d, c))
        nc.sync.dma_start(p_f[:], _img_view(p_d, c))

        Ib = pBf.tile([P, NB * HW_], BF16, tag="Ib")
        pb = pBf.tile([P, NB * HW_], BF16, tag="pb")
        Ipb = pBf.tile([P, NB * HW_], BF16, tag="Ipb")
        IIb = pBf.tile([P, NB * HW_], BF16, tag="IIb")
        flatI = I_f[:].rearrange("p hb w -> p (hb w)")
        flatp = p_f[:].rearrange("p hb w -> p (hb w)")
        nc.vector.tensor_copy(Ib[:], flatI)
        nc.vector.tensor_copy(pb[:], flatp)
        nc.vector.tensor_mul(Ipb[:], Ib[:], pb[:])
        nc.vector.tensor_mul(IIb[:], Ib[:], Ib[:])

        # stage 1: strided box2d sums via two banded indicator matmuls
        q = pQ.tile([P, 4, NS], F32, tag="q")       # [h's, tensor, w's]
        ybs = []
        for t, Xb in enumerate((Ib, pb, Ipb, IIb)):
            y = pY.tile([P, NB, NS], F32, tag="y")  # [w(chunk i), i, h's]
            for i in range(NB):
                for j in range(NB):
                    s0, s1 = _samp_range(j)
                    nc.tensor.matmul(
                        y[:, i, s0:s1],
                        lhsT=Xb[:, j * HW_ + i * P: j * HW_ + i * P + P],
                        rhs=bandS[:, j * NS + s0: j * NS + s1],
                        start=(j == 0), stop=(j == NB - 1))
            yb = pYb.tile([P, NB * NS], BF16, tag=f"yb{t}", name=f"yb{t}")
            nc.scalar.copy(yb[:], y[:].rearrange("p i s -> p (i s)"))
            ybs.append(yb)
        for t in range(4):
            for i in range(NB):
                s0, s1 = _samp_range(i)
                nc.tensor.matmul(
                    q[:, t, s0:s1],
                    lhsT=ybs[t][:, i * NS:(i + 1) * NS],
                    rhs=bandS[:, i * NS + s0: i * NS + s1],
                    start=(i == 0), stop=(i == NB - 1))

        # stage 2: tiny [128,128] elementwise in f32
        mI = pS2.tile([NS, NS], F32, tag="mI")
        mp = pS2.tile([NS, NS], F32, tag="mp")
        mIp = pS2.tile([NS, NS], F32, tag="mIp")
        mII = pS2.tile([NS, NS], F32, tag="mII")
        nc.vector.tensor_mul(mI[:], q[:, 0, :], normC[:])
        nc.vector.tensor_mul(mp[:], q[:, 1, :], normC[:])
        nc.vector.tensor_mul(mIp[:], q[:, 2, :], normC[:])
        nc.vector.tensor_mul(mII[:], q[:, 3, :], normC[:])
        u = pS2.tile([NS, NS], F32, tag="u")
        cov = pS2.tile([NS, NS], F32, tag="cov")
        vv = pS2.tile([NS, NS], F32, tag="vv")
        den = pS2.tile([NS, NS], F32, tag="den")
        rcp = pS2.tile([NS, NS], F32, tag="rcp")
        a_b = pS2.tile([NS, NS], BF16, tag="a_b")
        t2 = pS2.tile([NS, NS], F32, tag="t2")
        b_b = pS2.tile([NS, NS], BF16, tag="b_b")
        nc.vector.tensor_mul(u[:], mI[:], mp[:])
        nc.vector.tensor_sub(cov[:], mIp[:], u[:])
        nc.vector.tensor_mul(vv[:], mI[:], mI[:])
        nc.vector.scalar_tensor_tensor(
            den[:], mII[:], EPS, vv[:], op0=ALU.add, op1=ALU.subtract)
        nc.vector.reciprocal_approx_fast(rcp[:], den[:])
        nc.vector.tensor_mul(a_b[:], cov[:], rcp[:])
        nc.vector.tensor_mul(t2[:], a_b[:], mI[:])
        nc.vector.tensor_sub(b_b[:], mp[:], t2[:])

        # stage 3: dense mean_a/mean_b from strided samples
        o1a = pO1.tile([NS, HW_], F32, tag="o1a")
        o1b = pO1.tile([NS, HW_], F32, tag="o1b")
        nc.tensor.matmul(o1a[:], lhsT=a_b[:], rhs=W1[:], start=True, stop=True)
        nc.tensor.matmul(o1b[:], lhsT=b_b[:], rhs=W1[:], start=True, stop=True)
        t1a = pT1.tile([NS, HW_], BF16, tag="t1a")
        t1b = pT1.tile([NS, HW_], BF16, tag="t1b")
        nc.scalar.copy(t1a[:], o1a[:])
        nc.scalar.copy(t1b[:], o1b[:])

        ma = pMa.tile([P, NB * HW_], BF16, tag="ma")
        mb = pMa.tile([P, NB * HW_], F32, tag="mb")
        for j in range(NB):
            ra = pRR.tile([P, HW_], F32, tag="ra")
            rb = pRR.tile([P, HW_], F32, tag="rb")
            nc.tensor.matmul(ra[:], lhsT=t1a[:, j * P:(j + 1) * P], rhs=W2[:],
                             start=True, stop=True)
            nc.tensor.matmul(rb[:], lhsT=t1b[:, j * P:(j + 1) * P], rhs=W2[:],
                             start=True, stop=True)
            s = invn4[:, j:j + 1]
            sl = slice(j * HW_, (j + 1) * HW_)
            nc.scalar.mul(ma[:, sl], ra[:], s)
            nc.scalar.mul(mb[:, sl], rb[:], s)

        out_t = pOut.tile([P, NB, HW_], F32, tag="out")
        flato = out_t[:].rearrange("p hb w -> p (hb w)")
        t3 = pOut.tile([P, NB * HW_], BF16, tag="t3")
        nc.vector.tensor_mul(t3[:], ma[:], Ib[:])
        nc.vector.tensor_add(flato, t3[:], mb[:])

        nc.sync.dma_start(_img_view(out_d, c), out_t[:])


_NC_CACHE = None
LAST_RESULT = None


def _get_model():
    global _NC_CACHE
    if _NC_CACHE is None:
        _NC_CACHE = build_model()
    return _NC_CACHE


def kernel(I, p, _trace=False):
    global LAST_RESULT
    I = np.asarray(I, dtype=np.float32)
    p = np.asarray(p, dtype=np.float32)
    B = I.shape[0]
    assert I.shape == (B, CH, HW_, HW_), I.shape
    nc = _get_model()
    consts = make_consts()
    in_maps = []
    for k in range(NCORES):
        m = {"I": np.ascontiguousarray(I[k]), "p": np.ascontiguousarray(p[k])}
        m.update(consts)
        in_maps.append(m)
    res = run_bass_kernel_spmd(nc, in_maps, core_ids=list(range(NCORES)),
                               trace=_trace)
    LAST_RESULT = res
    out = np.stack([res.results[k]["out"] for k in range(NCORES)], axis=0)
    return out.astype(np.float32)


if __name__ == "__main__":
    rng = np.random.default_rng(0)
    I = rng.random((8, CH, HW_, HW_), dtype=np.float32)
    p = rng.random((8, CH, HW_, HW_), dtype=np.float32)
    out = kernel(I, p)
    print("out", out.shape, out.dtype, float(out.mean()))


# revision 7
# speedup vs baseline: 1.9040x; 1.0929x over previous
"""Guided filter (r=40, eps=1e-3) on 8 Trainium2 NeuronCores.

Sharding: pure data-parallel over the batch dim (8 batches -> 8 cores).
Each core processes 3 channel-images of 512x512.

v4 design (strided stage-2):
  a = cov/(var+eps) and b are pointwise functions of 81x81-box-filtered
  fields, so they are smooth on the r=40 scale. Stage 1 computes the four
  box sums (I, p, Ip, II) over EXACT full-res windows but only at a
  stride-4 sample grid (h,w in {4k+2}), via two banded bf16 indicator
  matmuls (box over the partition dim + transpose each time; 0/1 band is
  exact in bf16). Stage 2 normalizes with an exact f32 normC = 1/(nh*nw)
  and runs the elementwise math in f32 on tiny [128,128] tiles.
  Stage 3 evaluates mean_a/mean_b DENSELY from the strided a,b samples:
  one banded matmul per direction whose 0/1 matrix is the 81-window
  indicator on the sample grid (W1), then one with 1/n4(w) folded (W2,
  bf16); the h-direction sample count 1/n4(h) is applied exactly in f32
  as the ACT drain scale. Final combine: out = mean_a*I + mean_b on DVE.

  PSUM->SBUF traffic is ~15 chunk-drains/image (vs 48 for the full-res
  all-matmul design); TensorE work is ~9k cols/image. The kernel is
  memory-bound (~10 MB/core HBM traffic).
"""

import sys
import numpy as np
import ml_dtypes
from contextlib import ExitStack

sys.path.insert(0, "/opt/trn_rl_repo")

import concourse.bass as bass
import concourse.tile as tile
from concourse import bacc, mybir
from concourse.bass_utils import run_bass_kernel_spmd

F32 = mybir.dt.float32
BF16 = mybir.dt.bfloat16
ALU = mybir.AluOpType

R = 40
EPS = 1e-3
HW_ = 512
NB = 4
CH = 3
P = 128
NCORES = 8
S4 = 4
OFF = 2
NS = HW_ // S4          # 128 samples per axis


def _samp_range(j):
    # samples s with grid(s)=4s+OFF within [128j-40, 128j+127+40]
    s0 = max(0, -(-(P * j - R - OFF) // S4))
    s1 = min(NS, (P * j + P - 1 + R - OFF) // S4 + 1)
    return s0, s1


def make_consts():
    idx = np.arange(HW_)
    n1d = (np.minimum(idx + R, HW_ - 1) - np.maximum(idx - R, 0) + 1).astype(
        np.float64)
    grid = np.arange(NS) * S4 + OFF

    # 0/1 band: bandS[k, j*NS+s] = 1 if |(j*128+k) - grid(s)| <= R
    kk = np.arange(P)
    bandS = np.zeros((P, NB * NS), dtype=ml_dtypes.bfloat16)
    for j in range(NB):
        m = (np.abs((j * P + kk)[:, None] - grid[None, :]) <= R)
        bandS[:, j * NS:(j + 1) * NS] = m.astype(ml_dtypes.bfloat16)

    normC = (1.0 / (n1d[grid][:, None] * n1d[grid][None, :])).astype(np.float32)

    W_ind = (np.abs(grid[:, None] - idx[None, :]) <= R)
    n4 = W_ind.sum(axis=0).astype(np.float64)
    W1 = W_ind.astype(ml_dtypes.bfloat16)
    W2 = (W_ind * (1.0 / n4)[None, :]).astype(ml_dtypes.bfloat16)
    invn4 = np.ascontiguousarray((1.0 / n4).reshape(NB, P).T.astype(np.float32))
    return {"bandS": np.ascontiguousarray(bandS), "normC": normC,
            "W1": np.ascontiguousarray(W1), "W2": np.ascontiguousarray(W2),
            "invn4": invn4}


def _img_view(dram_ap, c):
    return dram_ap[c].rearrange("(hb hp) w -> hp hb w", hp=P)


def build_model():
    nc = bacc.Bacc("TRN2", target_bir_lowering=False, debug=False,
                   num_devices=NCORES)
    I_d = nc.dram_tensor("I", [CH, HW_, HW_], BF16, kind="ExternalInput").ap()
    p_d = nc.dram_tensor("p", [CH, HW_, HW_], BF16, kind="ExternalInput").ap()
    bandS_d = nc.dram_tensor("bandS", [P, NB * NS], BF16,
                             kind="ExternalInput").ap()
    normC_d = nc.dram_tensor("normC", [NS, NS], F32, kind="ExternalInput").ap()
    W1_d = nc.dram_tensor("W1", [NS, HW_], BF16, kind="ExternalInput").ap()
    W2_d = nc.dram_tensor("W2", [NS, HW_], BF16, kind="ExternalInput").ap()
    invn4_d = nc.dram_tensor("invn4", [P, NB], F32, kind="ExternalInput").ap()
    out_d = nc.dram_tensor("out", [CH, HW_, HW_], F32,
                           kind="ExternalOutput").ap()

    with tile.TileContext(nc) as tc:
        with ExitStack() as ctx:
            build_kernel(ctx, tc, I_d, p_d, out_d, bandS_d, normC_d, W1_d,
                         W2_d, invn4_d)
    nc.compile()
    return nc


def build_kernel(ctx, tc, I_d, p_d, out_d, bandS_d, normC_d, W1_d, W2_d,
                 invn4_d):
    nc = tc.nc

    consts = ctx.enter_context(tc.tile_pool(name="consts", bufs=1))
    bandS = consts.tile_from(bandS_d)
    normC = consts.tile_from(normC_d)
    W1 = consts.tile_from(W1_d)
    W2 = consts.tile_from(W2_d)
    invn4 = consts.tile_from(invn4_d)

    pIf = ctx.enter_context(tc.tile_pool(name="If", bufs=2))
    pPf = ctx.enter_context(tc.tile_pool(name="Pf", bufs=2))
    pBf = ctx.enter_context(tc.tile_pool(name="bfp", bufs=2))
    pYb = ctx.enter_context(tc.tile_pool(name="ybp", bufs=2))
    pS2 = ctx.enter_context(tc.tile_pool(name="s2p", bufs=2))
    pT1 = ctx.enter_context(tc.tile_pool(name="t1p", bufs=2))
    pMa = ctx.enter_context(tc.tile_pool(name="map", bufs=2))
    pOut = ctx.enter_context(tc.tile_pool(name="outp", bufs=2))
    pY = ctx.enter_context(tc.tile_pool(name="psy", bufs=3, space="PSUM"))
    pQ = ctx.enter_context(tc.tile_pool(name="psq", bufs=1, space="PSUM"))
    pO1 = ctx.enter_context(tc.tile_pool(name="pso1", bufs=1, space="PSUM"))
    pRR = ctx.enter_context(tc.tile_pool(name="psrr", bufs=1, space="PSUM"))

    for c in range(CH):
        Ib3 = pIf.tile([P, NB, HW_], BF16, tag="If")
        pb3 = pPf.tile([P, NB, HW_], BF16, tag="pf")
        nc.sync.dma_start(Ib3[:], _img_view(I_d, c))
        nc.sync.dma_start(pb3[:], _img_view(p_d, c))
        Ib = Ib3[:].rearrange("p hb w -> p (hb w)")
        pb = pb3[:].rearrange("p hb w -> p (hb w)")

        Ipb = pBf.tile([P, NB * HW_], BF16, tag="Ipb")
        IIb = pBf.tile([P, NB * HW_], BF16, tag="IIb")
        nc.vector.tensor_mul(Ipb[:], Ib, pb)
        nc.vector.tensor_mul(IIb[:], Ib, Ib)

        # stage 1: strided box2d sums via two banded indicator matmuls
        q = pQ.tile([P, 4, NS], F32, tag="q")       # [h's, tensor, w's]
        ybs = []
        for t, Xb in enumerate((Ib, pb, Ipb[:], IIb[:])):
            y = pY.tile([P, NB, NS], F32, tag="y")  # [w(chunk i), i, h's]
            for i in range(NB):
                for j in range(NB):
                    s0, s1 = _samp_range(j)
                    nc.tensor.matmul(
                        y[:, i, s0:s1],
                        lhsT=Xb[:, j * HW_ + i * P: j * HW_ + i * P + P],
                        rhs=bandS[:, j * NS + s0: j * NS + s1],
                        start=(j == 0), stop=(j == NB - 1))
            yb = pYb.tile([P, NB * NS], BF16, tag=f"yb{t}", name=f"yb{t}")
            nc.scalar.copy(yb[:], y[:].rearrange("p i s -> p (i s)"))
            ybs.append(yb)
        for t in range(4):
            for i in range(NB):
                s0, s1 = _samp_range(i)
                nc.tensor.matmul(
                    q[:, t, s0:s1],
                    lhsT=ybs[t][:, i * NS:(i + 1) * NS],
                    rhs=bandS[:, i * NS + s0: i * NS + s1],
                    start=(i == 0), stop=(i == NB - 1))

        # stage 2: tiny [128,128] elementwise in f32
        mI = pS2.tile([NS, NS], F32, tag="mI")
        mp = pS2.tile([NS, NS], F32, tag="mp")
        mIp = pS2.tile([NS, NS], F32, tag="mIp")
        mII = pS2.tile([NS, NS], F32, tag="mII")
        nc.vector.tensor_mul(mI[:], q[:, 0, :], normC[:])
        nc.vector.tensor_mul(mp[:], q[:, 1, :], normC[:])
        nc.vector.tensor_mul(mIp[:], q[:, 2, :], normC[:])
        nc.vector.tensor_mul(mII[:], q[:, 3, :], normC[:])
        u = pS2.tile([NS, NS], F32, tag="u")
        cov = pS2.tile([NS, NS], F32, tag="cov")
        vv = pS2.tile([NS, NS], F32, tag="vv")
        den = pS2.tile([NS, NS], F32, tag="den")
        rcp = pS2.tile([NS, NS], F32, tag="rcp")
        a_b = pS2.tile([NS, NS], BF16, tag="a_b")
        t2 = pS2.tile([NS, NS], F32, tag="t2")
        b_b = pS2.tile([NS, NS], BF16, tag="b_b")
        nc.vector.tensor_mul(u[:], mI[:], mp[:])
        nc.vector.tensor_sub(cov[:], mIp[:], u[:])
        nc.vector.tensor_mul(vv[:], mI[:], mI[:])
        nc.vector.scalar_tensor_tensor(
            den[:], mII[:], EPS, vv[:], op0=ALU.add, op1=ALU.subtract)
        nc.vector.reciprocal_approx_fast(rcp[:], den[:])
        nc.vector.tensor_mul(a_b[:], cov[:], rcp[:])
        nc.vector.tensor_mul(t2[:], a_b[:], mI[:])
        nc.vector.tensor_sub(b_b[:], mp[:], t2[:])

        # stage 3: dense mean_a/mean_b from strided samples
        o1a = pO1.tile([NS, HW_], F32, tag="o1a")
        o1b = pO1.tile([NS, HW_], F32, tag="o1b")
        nc.tensor.matmul(o1a[:], lhsT=a_b[:], rhs=W1[:], start=True, stop=True)
        nc.tensor.matmul(o1b[:], lhsT=b_b[:], rhs=W1[:], start=True, stop=True)
        t1a = pT1.tile([NS, HW_], BF16, tag="t1a")
        t1b = pT1.tile([NS, HW_], BF16, tag="t1b")
        nc.scalar.copy(t1a[:], o1a[:])
        nc.scalar.copy(t1b[:], o1b[:])

        ma = pMa.tile([P, NB * HW_], BF16, tag="ma")
        mb = pMa.tile([P, NB * HW_], F32, tag="mb")
        for j in range(NB):
            ra = pRR.tile([P, HW_], F32, tag="ra")
            rb = pRR.tile([P, HW_], F32, tag="rb")
            nc.tensor.matmul(ra[:], lhsT=t1a[:, j * P:(j + 1) * P], rhs=W2[:],
                             start=True, stop=True)
            nc.tensor.matmul(rb[:], lhsT=t1b[:, j * P:(j + 1) * P], rhs=W2[:],
                             start=True, stop=True)
            s = invn4[:, j:j + 1]
            sl = slice(j * HW_, (j + 1) * HW_)
            nc.scalar.mul(ma[:, sl], ra[:], s)
            nc.scalar.mul(mb[:, sl], rb[:], s)

        out_t = pOut.tile([P, NB, HW_], F32, tag="out")
        flato = out_t[:].rearrange("p hb w -> p (hb w)")
        t3 = pOut.tile([P, NB * HW_], BF16, tag="t3")
        nc.vector.tensor_mul(t3[:], ma[:], Ib)
        nc.vector.tensor_add(flato, t3[:], mb[:])

        nc.sync.dma_start(_img_view(out_d, c), out_t[:])


_NC_CACHE = None
LAST_RESULT = None


def _get_model():
    global _NC_CACHE
    if _NC_CACHE is None:
        _NC_CACHE = build_model()
    return _NC_CACHE


def kernel(I, p, _trace=False):
    global LAST_RESULT
    I = np.asarray(I, dtype=np.float32)
    p = np.asarray(p, dtype=np.float32)
    B = I.shape[0]
    assert I.shape == (B, CH, HW_, HW_), I.shape
    nc = _get_model()
    consts = make_consts()
    Ib = I.astype(ml_dtypes.bfloat16)
    pb = p.astype(ml_dtypes.bfloat16)
    in_maps = []
    for k in range(NCORES):
        m = {"I": np.ascontiguousarray(Ib[k]), "p": np.ascontiguousarray(pb[k])}
        m.update(consts)
        in_maps.append(m)
    res = run_bass_kernel_spmd(nc, in_maps, core_ids=list(range(NCORES)),
                               trace=_trace)
    LAST_RESULT = res
    out = np.stack([res.results[k]["out"] for k in range(NCORES)], axis=0)
    return out.astype(np.float32)


if __name__ == "__main__":
    rng = np.random.default_rng(0)
    I = rng.random((8, CH, HW_, HW_), dtype=np.float32)
    p = rng.random((8, CH, HW_, HW_), dtype=np.float32)
    out = kernel(I, p)
    print("out", out.shape, out.dtype, float(out.mean()))


# revision 8
# speedup vs baseline: 2.4415x; 1.2823x over previous
"""Guided filter (r=40, eps=1e-3) on 8 Trainium2 NeuronCores.

Sharding: pure data-parallel over the batch dim (8 batches -> 8 cores).
Each core processes 3 channel-images of 512x512.

v4 design (strided stage-2):
  a = cov/(var+eps) and b are pointwise functions of 81x81-box-filtered
  fields, so they are smooth on the r=40 scale. Stage 1 computes the four
  box sums (I, p, Ip, II) over EXACT full-res windows but only at a
  stride-4 sample grid (h,w in {4k+2}), via two banded bf16 indicator
  matmuls (box over the partition dim + transpose each time; 0/1 band is
  exact in bf16). Stage 2 normalizes with an exact f32 normC = 1/(nh*nw)
  and runs the elementwise math in f32 on tiny [128,128] tiles.
  Stage 3 evaluates mean_a/mean_b DENSELY from the strided a,b samples:
  one banded matmul per direction whose 0/1 matrix is the 81-window
  indicator on the sample grid (W1), then one with 1/n4(w) folded (W2,
  bf16); the h-direction sample count 1/n4(h) is applied exactly in f32
  as the ACT drain scale. Final combine: out = mean_a*I + mean_b on DVE.

  PSUM->SBUF traffic is ~15 chunk-drains/image (vs 48 for the full-res
  all-matmul design); TensorE work is ~9k cols/image. The kernel is
  memory-bound (~10 MB/core HBM traffic).
"""

import sys
import numpy as np
import ml_dtypes
from contextlib import ExitStack

sys.path.insert(0, "/opt/trn_rl_repo")

import concourse.bass as bass
import concourse.tile as tile
from concourse import bacc, mybir
from concourse.bass_utils import run_bass_kernel_spmd

F32 = mybir.dt.float32
BF16 = mybir.dt.bfloat16
ALU = mybir.AluOpType

R = 40
EPS = 1e-3
HW_ = 512
NB = 4
CH = 3
P = 128
NCORES = 8
S4 = 4
OFF = 2
NS = HW_ // S4          # 128 samples per axis


def _samp_range(j):
    # samples s with grid(s)=4s+OFF within [128j-40, 128j+127+40]
    s0 = max(0, -(-(P * j - R - OFF) // S4))
    s1 = min(NS, (P * j + P - 1 + R - OFF) // S4 + 1)
    return s0, s1


def make_consts():
    idx = np.arange(HW_)
    n1d = (np.minimum(idx + R, HW_ - 1) - np.maximum(idx - R, 0) + 1).astype(
        np.float64)
    grid = np.arange(NS) * S4 + OFF

    # 0/1 band: bandS[k, j*NS+s] = 1 if |(j*128+k) - grid(s)| <= R
    kk = np.arange(P)
    bandS = np.zeros((P, NB * NS), dtype=ml_dtypes.bfloat16)
    for j in range(NB):
        m = (np.abs((j * P + kk)[:, None] - grid[None, :]) <= R)
        bandS[:, j * NS:(j + 1) * NS] = m.astype(ml_dtypes.bfloat16)

    normC = (1.0 / (n1d[grid][:, None] * n1d[grid][None, :])).astype(np.float32)

    W_ind = (np.abs(grid[:, None] - idx[None, :]) <= R)
    n4 = W_ind.sum(axis=0).astype(np.float64)
    W1 = W_ind.astype(ml_dtypes.bfloat16)
    W2 = (W_ind * (1.0 / n4)[None, :]).astype(ml_dtypes.bfloat16)
    invn4 = np.ascontiguousarray((1.0 / n4).reshape(NB, P).T.astype(np.float32))
    return {"bandS": np.ascontiguousarray(bandS), "normC": normC,
            "W1": np.ascontiguousarray(W1), "W2": np.ascontiguousarray(W2),
            "invn4": invn4}


def _img_view(dram_ap, c):
    return dram_ap[c].rearrange("(hb hp) w -> hp hb w", hp=P)


def build_model():
    nc = bacc.Bacc("TRN2", target_bir_lowering=False, debug=False,
                   num_devices=NCORES)
    I_d = nc.dram_tensor("I", [CH, HW_, HW_], BF16, kind="ExternalInput").ap()
    p_d = nc.dram_tensor("p", [CH, HW_, HW_], BF16, kind="ExternalInput").ap()
    bandS_d = nc.dram_tensor("bandS", [P, NB * NS], BF16,
                             kind="ExternalInput").ap()
    normC_d = nc.dram_tensor("normC", [NS, NS], F32, kind="ExternalInput").ap()
    W1_d = nc.dram_tensor("W1", [NS, HW_], BF16, kind="ExternalInput").ap()
    W2_d = nc.dram_tensor("W2", [NS, HW_], BF16, kind="ExternalInput").ap()
    invn4_d = nc.dram_tensor("invn4", [P, NB], F32, kind="ExternalInput").ap()
    out_d = nc.dram_tensor("out", [CH, HW_, HW_], F32,
                           kind="ExternalOutput").ap()

    with tile.TileContext(nc) as tc:
        with ExitStack() as ctx:
            build_kernel(ctx, tc, I_d, p_d, out_d, bandS_d, normC_d, W1_d,
                         W2_d, invn4_d)
    nc.compile()
    return nc


def build_kernel(ctx, tc, I_d, p_d, out_d, bandS_d, normC_d, W1_d, W2_d,
                 invn4_d):
    nc = tc.nc

    consts = ctx.enter_context(tc.tile_pool(name="consts", bufs=1))
    bandS = consts.tile_from(bandS_d)
    normC = consts.tile_from(normC_d)
    W1 = consts.tile_from(W1_d)
    W2 = consts.tile_from(W2_d)
    invn4 = consts.tile_from(invn4_d)

    pIf = ctx.enter_context(tc.tile_pool(name="If", bufs=3))
    pPf = ctx.enter_context(tc.tile_pool(name="Pf", bufs=3))
    pBf = ctx.enter_context(tc.tile_pool(name="bfp", bufs=3))
    pYb = ctx.enter_context(tc.tile_pool(name="ybp", bufs=2))
    pS2 = ctx.enter_context(tc.tile_pool(name="s2p", bufs=2))
    pT1 = ctx.enter_context(tc.tile_pool(name="t1p", bufs=2))
    pF1 = ctx.enter_context(tc.tile_pool(name="f1p", bufs=2))
    pOut = ctx.enter_context(tc.tile_pool(name="outp", bufs=2))
    pY = ctx.enter_context(tc.tile_pool(name="psy", bufs=2, space="PSUM"))
    pQ = ctx.enter_context(tc.tile_pool(name="psq", bufs=2, space="PSUM"))
    pO1 = ctx.enter_context(tc.tile_pool(name="pso1", bufs=1, space="PSUM"))
    pRR = ctx.enter_context(tc.tile_pool(name="psrr", bufs=1, space="PSUM"))

    # phase-batched emission across channels: engine queues are FIFO, so
    # emitting each phase for all channels keeps every engine fed.
    chan = {}
    for c in range(CH):
        Ib3 = pIf.tile([P, NB, HW_], BF16, tag="If", name=f"Ib3_{c}")
        pb3 = pPf.tile([P, NB, HW_], BF16, tag="pf", name=f"pb3_{c}")
        nc.sync.dma_start(Ib3[:], _img_view(I_d, c))
        nc.sync.dma_start(pb3[:], _img_view(p_d, c))
        chan[c] = {"Ib3": Ib3, "pb3": pb3}

    for c in range(CH):
        d = chan[c]
        Ib = d["Ib3"][:].rearrange("p hb w -> p (hb w)")
        pb = d["pb3"][:].rearrange("p hb w -> p (hb w)")
        Ipb = pBf.tile([P, NB * HW_], BF16, tag="Ipb", name=f"Ipb_{c}")
        IIb = pBf.tile([P, NB * HW_], BF16, tag="IIb", name=f"IIb_{c}")
        nc.vector.tensor_mul(Ipb[:], Ib, pb)
        nc.vector.tensor_mul(IIb[:], Ib, Ib)
        d["Ipb"], d["IIb"] = Ipb, IIb
        d["Ib"], d["pb"] = Ib, pb

    for c in range(CH):
        d = chan[c]
        q = pQ.tile([P, 4, NS], F32, tag="q", name=f"q_{c}")
        ybs = []
        for t, Xb in enumerate((d["Ib"], d["pb"], d["Ipb"][:], d["IIb"][:])):
            y = pY.tile([P, NB, NS], F32, tag="y", name=f"y_{c}_{t}")
            for i in range(NB):
                for j in range(NB):
                    s0, s1 = _samp_range(j)
                    nc.tensor.matmul(
                        y[:, i, s0:s1],
                        lhsT=Xb[:, j * HW_ + i * P: j * HW_ + i * P + P],
                        rhs=bandS[:, j * NS + s0: j * NS + s1],
                        start=(j == 0), stop=(j == NB - 1))
            yb = pYb.tile([P, NB * NS], BF16, tag=f"yb{t}", name=f"yb{t}_{c}")
            nc.scalar.copy(yb[:], y[:].rearrange("p i s -> p (i s)"))
            ybs.append(yb)
        for t in range(4):
            for i in range(NB):
                s0, s1 = _samp_range(i)
                nc.tensor.matmul(
                    q[:, t, s0:s1],
                    lhsT=ybs[t][:, i * NS:(i + 1) * NS],
                    rhs=bandS[:, i * NS + s0: i * NS + s1],
                    start=(i == 0), stop=(i == NB - 1))
        d["q"] = q

    for c in range(CH):
        d = chan[c]
        q = d["q"]
        mI = pS2.tile([NS, NS], F32, tag="mI", name=f"mI_{c}")
        mp = pS2.tile([NS, NS], F32, tag="mp", name=f"mp_{c}")
        mIp = pS2.tile([NS, NS], F32, tag="mIp", name=f"mIp_{c}")
        mII = pS2.tile([NS, NS], F32, tag="mII", name=f"mII_{c}")
        nc.vector.tensor_mul(mI[:], q[:, 0, :], normC[:])
        nc.vector.tensor_mul(mp[:], q[:, 1, :], normC[:])
        nc.vector.tensor_mul(mIp[:], q[:, 2, :], normC[:])
        nc.vector.tensor_mul(mII[:], q[:, 3, :], normC[:])
        u = pS2.tile([NS, NS], F32, tag="u", name=f"u_{c}")
        cov = pS2.tile([NS, NS], F32, tag="cov", name=f"cov_{c}")
        vv = pS2.tile([NS, NS], F32, tag="vv", name=f"vv_{c}")
        den = pS2.tile([NS, NS], F32, tag="den", name=f"den_{c}")
        rcp = pS2.tile([NS, NS], F32, tag="rcp", name=f"rcp_{c}")
        a_b = pS2.tile([NS, NS], BF16, tag="a_b", name=f"a_b_{c}")
        t2 = pS2.tile([NS, NS], F32, tag="t2", name=f"t2_{c}")
        b_b = pS2.tile([NS, NS], BF16, tag="b_b", name=f"b_b_{c}")
        nc.vector.tensor_mul(u[:], mI[:], mp[:])
        nc.vector.tensor_sub(cov[:], mIp[:], u[:])
        nc.vector.tensor_mul(vv[:], mI[:], mI[:])
        nc.vector.scalar_tensor_tensor(
            den[:], mII[:], EPS, vv[:], op0=ALU.add, op1=ALU.subtract)
        nc.vector.reciprocal_approx_fast(rcp[:], den[:])
        nc.vector.tensor_mul(a_b[:], cov[:], rcp[:])
        nc.vector.tensor_mul(t2[:], a_b[:], mI[:])
        nc.vector.tensor_sub(b_b[:], mp[:], t2[:])
        d["a_b"], d["b_b"] = a_b, b_b

    for c in range(CH):
        d = chan[c]
        o1a = pO1.tile([NS, HW_], F32, tag="o1a", name=f"o1a_{c}")
        o1b = pO1.tile([NS, HW_], F32, tag="o1b", name=f"o1b_{c}")
        nc.tensor.matmul(o1a[:], lhsT=d["a_b"][:], rhs=W1[:], start=True,
                         stop=True)
        nc.tensor.matmul(o1b[:], lhsT=d["b_b"][:], rhs=W1[:], start=True,
                         stop=True)
        t1a = pT1.tile([NS, HW_], BF16, tag="t1a", name=f"t1a_{c}")
        t1b = pT1.tile([NS, HW_], BF16, tag="t1b", name=f"t1b_{c}")
        nc.scalar.copy(t1a[:], o1a[:])
        nc.scalar.copy(t1b[:], o1b[:])

        out_t = pOut.tile([P, NB, HW_], F32, tag="out", name=f"out_{c}")
        for j in range(NB):
            ra = pRR.tile([P, HW_], F32, tag="ra", name=f"ra_{c}_{j}")
            rb = pRR.tile([P, HW_], F32, tag="rb", name=f"rb_{c}_{j}")
            nc.tensor.matmul(ra[:], lhsT=t1a[:, j * P:(j + 1) * P], rhs=W2[:],
                             start=True, stop=True)
            nc.tensor.matmul(rb[:], lhsT=t1b[:, j * P:(j + 1) * P], rhs=W2[:],
                             start=True, stop=True)
            s = invn4[:, j:j + 1]
            f1 = pF1.tile([P, HW_], F32, tag="f1", name=f"f1_{c}_{j}")
            nc.vector.scalar_tensor_tensor(
                f1[:], ra[:], s, d["Ib3"][:, j, :], op0=ALU.mult, op1=ALU.mult)
            nc.vector.scalar_tensor_tensor(
                out_t[:, j, :], rb[:], s, f1[:], op0=ALU.mult, op1=ALU.add)

        nc.sync.dma_start(_img_view(out_d, c), out_t[:])


_NC_CACHE = None
LAST_RESULT = None


def _get_model():
    global _NC_CACHE
    if _NC_CACHE is None:
        _NC_CACHE = build_model()
    return _NC_CACHE


def kernel(I, p, _trace=False):
    global LAST_RESULT
    I = np.asarray(I, dtype=np.float32)
    p = np.asarray(p, dtype=np.float32)
    B = I.shape[0]
    assert I.shape == (B, CH, HW_, HW_), I.shape
    nc = _get_model()
    consts = make_consts()
    Ib = I.astype(ml_dtypes.bfloat16)
    pb = p.astype(ml_dtypes.bfloat16)
    in_maps = []
    for k in range(NCORES):
        m = {"I": np.ascontiguousarray(Ib[k]), "p": np.ascontiguousarray(pb[k])}
        m.update(consts)
        in_maps.append(m)
    res = run_bass_kernel_spmd(nc, in_maps, core_ids=list(range(NCORES)),
                               trace=_trace)
    LAST_RESULT = res
    out = np.stack([res.results[k]["out"] for k in range(NCORES)], axis=0)
    return out.astype(np.float32)


if __name__ == "__main__":
    rng = np.random.default_rng(0)
    I = rng.random((8, CH, HW_, HW_), dtype=np.float32)
    p = rng.random((8, CH, HW_, HW_), dtype=np.float32)
    out = kernel(I, p)
    print("out", out.shape, out.dtype, float(out.mean()))
